# revision 4
# baseline (speedup 1.0000x reference)
"""Bass/Trainium2 kernel for nn_BigramLanguageModel: 8-core SPMD.

Sharding: tokens 8-way (core = batch_row*2 + seq_half) through embedding +
8 transformer layers with a per-pair KV AllGather per layer; vocab 8-way for
the LM head (xf AllGather over all 8 cores); per-core partial sumexp, host
combines the CE loss. Host folds LN gains into weights, casts to bf16, and
does the embedding gather.
"""
import os
import numpy as np
import ml_dtypes

import concourse.bacc as bacc
import concourse.tile as tile
from concourse import mybir
from concourse.masks import make_identity

F32 = mybir.dt.float32
BF16 = mybir.dt.bfloat16

N_CORES = 8
B, T, D, H, L_FULL, V = 4, 1024, 1024, 16, 8, 50257
DH = D // H          # 64
FF = 4 * D           # 4096
TLOC = 512           # tokens per core
NT = TLOC // 128     # 4 local token tiles
ND = D // 128        # 8
EPS = 1e-5
VS = 6656            # vocab shard per core (13*512), 8*VS = 53248 >= V
VCH = VS // 512      # 13
NTG = (B * T) // 128  # 32 global token tiles

L = int(os.environ.get("KBENCH_NLAYERS", L_FULL))

_CACHE = {}


def _build_program():
    nc = bacc.Bacc(None, target_bir_lowering=False, num_devices=N_CORES)

    x0_d = nc.dram_tensor("x0", [TLOC, D], F32, kind="ExternalInput")
    m01_d = nc.dram_tensor("m01", [NT, 128, T], BF16, kind="ExternalInput")
    vmask_d = nc.dram_tensor("vmask", [128, VCH, 512], BF16, kind="ExternalInput")
    wq_d = nc.dram_tensor("wq", [L, D, D], BF16, kind="ExternalInput")
    wk_d = nc.dram_tensor("wk", [L, D, D], BF16, kind="ExternalInput")
    wv_d = nc.dram_tensor("wv", [L, D, D], BF16, kind="ExternalInput")
    wo_d = nc.dram_tensor("wo", [L, D, D], BF16, kind="ExternalInput")
    w1_d = nc.dram_tensor("w1", [L, D, FF], BF16, kind="ExternalInput")
    w2_d = nc.dram_tensor("w2", [L, FF, D], BF16, kind="ExternalInput")
    wlm_d = nc.dram_tensor("wlm", [D, VS], BF16, kind="ExternalInput")

    logits_d = nc.dram_tensor("logits", [B * T, VS], F32, kind="ExternalOutput")
    se_d = nc.dram_tensor("sumexp", [128, NTG], F32, kind="ExternalOutput")

    with tile.TileContext(nc) as tc:
        with (
            tc.tile_pool(name="state", bufs=1) as state,
            tc.tile_pool(name="acts", bufs=1) as acts,
            tc.tile_pool(name="hbuf", bufs=2) as hbuf,
            tc.tile_pool(name="wpool", bufs=7) as wpool,
            tc.tile_pool(name="small", bufs=4) as small,
            tc.tile_pool(name="probs_p", bufs=2) as probs_p,
            tc.tile_pool(name="pt_p", bufs=2) as pt_p,
            tc.tile_pool(name="attn_p", bufs=2) as attn_p,
            tc.tile_pool(name="lgp", bufs=4) as lgp,
            tc.tile_pool(name="escp", bufs=2) as escp,
            tc.tile_pool(name="ps_g", bufs=2, space="PSUM") as ps_g,
            tc.tile_pool(name="ps_tp", bufs=2, space="PSUM") as ps_tp,
            tc.tile_pool(name="ps_at", bufs=2, space="PSUM") as ps_at,
            tc.tile_pool(name="ps_sc", bufs=2, space="PSUM") as ps_sc,
            tc.tile_pool(name="dram", bufs=2, space="DRAM") as dram,
        ):
            ident = state.tile([128, 128], BF16)
            make_identity(nc, ident)
            eps_t = state.tile([128, 1], F32)
            nc.vector.memset(eps_t, EPS)

            # resident state: x (fp32 residual), causal masks
            x = state.tile([128, NT, D], F32)
            for k in range(NT):
                nc.sync.dma_start(x[:, k], x0_d[k * 128:(k + 1) * 128, :])
            m01 = state.tile([128, NT, T], BF16)
            nc.sync.dma_start(m01, m01_d[:].rearrange("k p t -> p k t"))

            def layernorm_cast(src_f32, dst_bf16):
                """dst = (src - mean)/std per partition row, cast to bf16."""
                stats = small.tile([128, 2, 6], F32, tag="stats", name="stats")
                xg = src_f32.rearrange("p (s f) -> p s f", s=2)
                for s in range(2):
                    nc.vector.bn_stats(out=stats[:, s], in_=xg[:, s])
                mv = small.tile([128, 2], F32, tag="mv", name="mv")
                nc.vector.bn_aggr(out=mv, in_=stats)
                std = small.tile([128, 1], F32, tag="std", name="std")
                nc.scalar.activation(out=std, in_=mv[:, 1:2],
                                     func=mybir.ActivationFunctionType.Sqrt,
                                     bias=eps_t)
                rstd = small.tile([128, 1], F32, tag="rstd", name="rstd")
                nc.vector.reciprocal(out=rstd, in_=std)
                nmr = small.tile([128, 1], F32, tag="nmr", name="nmr")
                nc.vector.tensor_scalar(out=nmr, in0=mv[:, 0:1], scalar1=rstd,
                                        scalar2=-1.0,
                                        op0=mybir.AluOpType.mult,
                                        op1=mybir.AluOpType.mult)
                nc.scalar.activation(out=dst_bf16, in_=src_f32,
                                     func=mybir.ActivationFunctionType.Identity,
                                     bias=nmr, scale=rstd)

            def transpose_block(dst, src):
                """dst[128,128] (SBUF bf16) = src[128,128].T via PE+ACT."""
                pt = ps_tp.tile([128, 128], BF16, tag="tp", name="tp")
                nc.tensor.transpose(pt, src, ident)
                nc.scalar.copy(dst, pt)

            def load_w(dram_ap, tag):
                """Load [1024rows x 512cols] weight slab as [128, nsub, 512]."""
                rows = dram_ap.shape[0]
                nsub = rows // 128
                t = wpool.tile([128, nsub, 512], BF16, tag="w", name=f"w_{tag}")
                nc.sync.dma_start(t, dram_ap.rearrange("(s p) n -> p s n", p=128))
                return t

            for l in range(L):
                # ---- LN1 + h^T
                hT = acts.tile([128, ND, TLOC], BF16, tag="hT", name="hT")
                for k in range(NT):
                    h = hbuf.tile([128, D], BF16, tag="h", name="h")
                    layernorm_cast(x[:, k], h)
                    for d in range(ND):
                        transpose_block(hT[:, d, k * 128:(k + 1) * 128],
                                        h[:, d * 128:(d + 1) * 128])

                # ---- QKV projections
                wq_t = [load_w(wq_d[l, :, nh * 512:(nh + 1) * 512], "wa") for nh in range(2)]
                wk_t = [load_w(wk_d[l, :, nh * 512:(nh + 1) * 512], "wb") for nh in range(2)]
                wv_t = [load_w(wv_d[l, :, nh * 512:(nh + 1) * 512], "wc") for nh in range(2)]

                QT = acts.tile([128, ND, TLOC], BF16, tag="QT", name="QT")
                KTf = acts.tile([128, ND, T], BF16, tag="KTf", name="KTf")
                Vf = acts.tile([128, T // 128, D], BF16, tag="Vf", name="Vf")
                for (wt, dst) in ((wq_t, QT), (wk_t, None)):
                    for j in range(ND):  # output n-chunk of 128
                        ps = ps_g.tile([128, 512], F32, tag="g", name="psg")
                        for d in range(ND):
                            nc.tensor.matmul(
                                ps, wt[j // 4][:, d, (j % 4) * 128:(j % 4) * 128 + 128],
                                hT[:, d, :], start=(d == 0), stop=(d == ND - 1))
                        if dst is None:
                            nc.scalar.copy(KTf[:, j, 0:512], ps)
                        else:
                            nc.scalar.copy(dst[:, j, :], ps)
                for k in range(NT):
                    for nh in range(2):
                        ps = ps_g.tile([128, 512], F32, tag="g", name="psg")
                        for d in range(ND):
                            nc.tensor.matmul(
                                ps, hT[:, d, k * 128:(k + 1) * 128],
                                wv_t[nh][:, d, :], start=(d == 0), stop=(d == ND - 1))
                        nc.scalar.copy(Vf[:, k, nh * 512:(nh + 1) * 512], ps)

                # ---- KV AllGather within the pair
                cc_in = dram.tile([2, 128, 4096], BF16, tag="kv_in", name="kv_in")
                cc_out = dram.tile([2, 2, 128, 4096], BF16, tag="kv_out", name="kv_out")
                nc.sync.dma_start(cc_in[0].rearrange("p (s n) -> p s n", s=8),
                                  KTf[:, :, 0:512])
                nc.sync.dma_start(cc_in[1].rearrange("p (s n) -> p s n", s=4),
                                  Vf[:, 0:NT, :])
                nc.gpsimd.collective_compute(
                    "AllGather", mybir.AluOpType.bypass,
                    replica_groups=[[0, 1], [2, 3], [4, 5], [6, 7]],
                    ins=[cc_in.opt()], outs=[cc_out.opt()])
                for r in range(2):
                    nc.sync.dma_start(
                        KTf[:, :, r * 512:(r + 1) * 512],
                        cc_out[r, 0].rearrange("p (s n) -> p s n", s=8))
                    nc.sync.dma_start(
                        Vf[:, r * 4:(r + 1) * 4, :],
                        cc_out[r, 1].rearrange("p (s n) -> p s n", s=4))

                wo_t = [load_w(wo_d[l, :, nh * 512:(nh + 1) * 512], "wa") for nh in range(2)]

                # ---- attention (uniform full-span, causality via m01 data)
                attT = acts.tile([128, ND, TLOC], BF16, tag="attT", name="attT")
                for k in range(NT):
                    attn = attn_p.tile([128, H, DH], BF16, tag="attn", name="attn")
                    for hd in range(H):
                        po = (hd % 2) * 64
                        j = hd // 2
                        probs = probs_p.tile([128, T], BF16, tag="probs", name="probs")
                        for c in range(2):
                            sc = ps_sc.tile([128, 512], F32, tag="sc", name="sc")
                            nc.tensor.matmul(
                                sc, QT[po:po + 64, j, k * 128:(k + 1) * 128],
                                KTf[po:po + 64, j, c * 512:(c + 1) * 512],
                                start=True, stop=True)
                            nc.scalar.activation(
                                out=probs[:, c * 512:(c + 1) * 512], in_=sc,
                                func=mybir.ActivationFunctionType.Exp)
                        nc.vector.tensor_mul(probs, probs, m01[:, k, :])
                        se = small.tile([128, 1], F32, tag="se", name="se")
                        nc.vector.reduce_sum(se, probs, axis=mybir.AxisListType.X)
                        rcp = small.tile([128, 1], F32, tag="rcp", name="rcp")
                        nc.vector.reciprocal(out=rcp, in_=se)
                        pT = pt_p.tile([128, T // 128, 128], BF16, tag="pT", name="pT")
                        for s in range(T // 128):
                            ptp = ps_tp.tile([128, 128], BF16, tag="tp", name="tp2")
                            nc.tensor.transpose(
                                ptp, probs[:, s * 128:(s + 1) * 128], ident)
                            nc.vector.tensor_copy(pT[:, s, :], ptp)
                        pa = ps_at.tile([128, DH], F32, tag="at", name="pat")
                        for s in range(T // 128):
                            nc.tensor.matmul(
                                pa, pT[:, s, :], Vf[:, s, hd * DH:(hd + 1) * DH],
                                start=(s == 0), stop=(s == T // 128 - 1))
                        nc.vector.tensor_scalar_mul(attn[:, hd, :], in0=pa, scalar1=rcp)
                    av = attn.rearrange("p h d -> p (h d)")
                    for d in range(ND):
                        transpose_block(attT[:, d, k * 128:(k + 1) * 128],
                                        av[:, d * 128:(d + 1) * 128])

                # ---- output projection + residual
                for k in range(NT):
                    for nh in range(2):
                        ps = ps_g.tile([128, 512], F32, tag="g", name="psg")
                        for d in range(ND):
                            nc.tensor.matmul(
                                ps, attT[:, d, k * 128:(k + 1) * 128],
                                wo_t[nh][:, d, :], start=(d == 0), stop=(d == ND - 1))
                        xs = x[:, k, nh * 512:(nh + 1) * 512]
                        nc.vector.tensor_add(xs, xs, ps)

                # ---- LN2 + FFN (two 2048-halves to bound SBUF)
                h2T = acts.tile([128, ND, TLOC], BF16, tag="hT", name="h2T")
                for k in range(NT):
                    h2 = hbuf.tile([128, D], BF16, tag="h", name="h2")
                    layernorm_cast(x[:, k], h2)
                    for d in range(ND):
                        transpose_block(h2T[:, d, k * 128:(k + 1) * 128],
                                        h2[:, d * 128:(d + 1) * 128])
                for half in range(2):
                    w1_t = [load_w(w1_d[l, :, (half * 4 + q) * 512:(half * 4 + q + 1) * 512], "wa")
                            for q in range(4)]
                    uT = acts.tile([128, 16, TLOC], BF16, tag="uT", name="uT")
                    for m in range(16):  # ff chunk of 128 within this half
                        ps = ps_g.tile([128, 512], F32, tag="g", name="psg")
                        for d in range(ND):
                            nc.tensor.matmul(
                                ps, w1_t[m // 4][:, d, (m % 4) * 128:(m % 4) * 128 + 128],
                                h2T[:, d, :],
                                start=(d == 0), stop=(d == ND - 1))
                        nc.scalar.activation(out=uT[:, m, :], in_=ps,
                                             func=mybir.ActivationFunctionType.Relu)
                    w2_t = [load_w(w2_d[l, half * 2048 + g * 1024:half * 2048 + (g + 1) * 1024,
                                        nh * 512:(nh + 1) * 512], f"w2_{g}_{nh}")
                            for g in range(2) for nh in range(2)]
                    for k in range(NT):
                        for nh in range(2):
                            ps = ps_g.tile([128, 512], F32, tag="g", name="psg")
                            for s in range(16):
                                wt = w2_t[(s // 8) * 2 + nh]
                                nc.tensor.matmul(
                                    ps, uT[:, s, k * 128:(k + 1) * 128],
                                    wt[:, s % 8, :], start=(s == 0), stop=(s == 15))
                            xs = x[:, k, nh * 512:(nh + 1) * 512]
                            nc.vector.tensor_add(xs, xs, ps)

            # ---- final LN + xf AllGather
            xfT = acts.tile([128, ND, TLOC], BF16, tag="hT", name="xfT")
            for k in range(NT):
                hf = hbuf.tile([128, D], BF16, tag="h", name="hf")
                layernorm_cast(x[:, k], hf)
                for d in range(ND):
                    transpose_block(xfT[:, d, k * 128:(k + 1) * 128],
                                    hf[:, d * 128:(d + 1) * 128])
            xf_in = dram.tile([128, ND, TLOC], BF16, tag="xf_in", name="xf_in")
            xf_out = dram.tile([N_CORES, 128, ND, TLOC], BF16, tag="xf_out", name="xf_out")
            nc.sync.dma_start(xf_in, xfT)
            nc.gpsimd.collective_compute(
                "AllGather", mybir.AluOpType.bypass,
                replica_groups=[[0, 1, 2, 3, 4, 5, 6, 7]],
                ins=[xf_in.opt()], outs=[xf_out.opt()])
            # ---- LM head over the vocab shard: two vocab groups, per-core xf
            vm = acts.tile([128, VCH, 512], BF16, tag="Vf", name="vm")
            nc.sync.dma_start(vm, vmask_d[:])
            separts = state.tile([128, NTG, VCH], F32)
            for g0, g1 in ((0, 7), (7, VCH)):
                wls = [load_w(wlm_d[:, vc * 512:(vc + 1) * 512], f"wlm{vc}")
                       for vc in range(g0, g1)]
                for r in range(N_CORES):
                    xfr = acts.tile([128, ND, TLOC], BF16,
                                    tag=("QT" if r % 2 == 0 else "hT"), name="xfr")
                    nc.sync.dma_start(xfr, xf_out[r])
                    for kk in range(NT):
                        tt = r * NT + kk
                        for i, vc in enumerate(range(g0, g1)):
                            ps = ps_g.tile([128, 512], F32, tag="g", name="psg")
                            for d in range(ND):
                                nc.tensor.matmul(
                                    ps, xfr[:, d, kk * 128:(kk + 1) * 128],
                                    wls[i][:, d, :], start=(d == 0), stop=(d == ND - 1))
                            lg = lgp.tile([128, 512], F32, tag="lg", name="lg")
                            nc.scalar.copy(lg, ps)
                            esc = escp.tile([128, 512], BF16, tag="esc", name="esc")
                            nc.scalar.activation(out=esc, in_=ps,
                                                 func=mybir.ActivationFunctionType.Exp)
                            nc.vector.tensor_mul(esc, esc, vm[:, vc, :])
                            nc.vector.reduce_sum(separts[:, tt, vc:vc + 1], esc,
                                                 axis=mybir.AxisListType.X)
                            nc.sync.dma_start(
                                logits_d[tt * 128:(tt + 1) * 128,
                                         vc * 512:(vc + 1) * 512], lg)
            sesum = state.tile([128, NTG], F32)
            for tt in range(NTG):
                nc.vector.reduce_sum(sesum[:, tt:tt + 1], separts[:, tt],
                                     axis=mybir.AxisListType.X)
            nc.sync.dma_start(se_d[:], sesum)

    nc.compile()
    return nc


def _prep_inputs(idx, targets, tok_emb, pos_emb, Wq, Wk, Wv, Wo, bo,
                 ln1_g, ln1_b, ln2_g, ln2_b, W1, b1, W2, b2, lnf_g, lnf_b,
                 Wlm, blm):
    """Host-side: embedding gather, LN-gain folding, bf16 casts, masks."""
    f32 = np.float32
    bf = ml_dtypes.bfloat16
    for b_ in (bo, ln1_b, ln2_b, b1, b2, lnf_b, blm):
        assert np.max(np.abs(np.asarray(b_))) == 0.0, "nonzero bias unsupported"

    x0 = np.asarray(tok_emb, f32)[np.asarray(idx)] + np.asarray(pos_emb, f32)[None, :T]
    x0 = x0.reshape(B * T, D)

    scale = np.float32(D ** -0.5)
    g1 = np.asarray(ln1_g, f32)[:, :, None]
    g2 = np.asarray(ln2_g, f32)[:, :, None]
    wq = (np.asarray(Wq, f32) * g1 * scale).astype(bf)
    wk = (np.asarray(Wk, f32) * g1).astype(bf)
    wv = (np.asarray(Wv, f32) * g1).astype(bf)
    wo = np.asarray(Wo, f32).astype(bf)
    w1 = (np.asarray(W1, f32) * g2).astype(bf)
    w2 = np.asarray(W2, f32).astype(bf)
    wlm_full = (np.asarray(Wlm, f32) * np.asarray(lnf_g, f32)[:, None]).astype(bf)
    wlm_pad = np.zeros((D, N_CORES * VS), bf)
    wlm_pad[:, :V] = wlm_full

    in_maps = []
    for c in range(N_CORES):
        half = c % 2
        # causal 0/1 masks for this core's token tiles
        m01 = np.zeros((NT, 128, T), f32)
        for k in range(NT):
            gt = half * TLOC + k * 128 + np.arange(128)[:, None]
            m01[k] = (np.arange(T)[None, :] <= gt)
        vmask = np.zeros((VCH, 512), f32)
        base = c * VS
        for vc in range(VCH):
            cols = base + vc * 512 + np.arange(512)
            vmask[vc] = (cols < V)
        in_maps.append(dict(
            x0=np.ascontiguousarray(x0[c * TLOC:(c + 1) * TLOC]),
            m01=m01.astype(bf),
            vmask=np.broadcast_to(vmask.astype(bf), (128, VCH, 512)).copy(),
            wq=wq[:L], wk=wk[:L], wv=wv[:L], wo=wo[:L],
            w1=w1[:L], w2=w2[:L],
            wlm=np.ascontiguousarray(wlm_pad[:, c * VS:(c + 1) * VS]),
        ))
    return in_maps


def _get_exec():
    """Build program once; return a callable(in_maps) -> per-core out dicts."""
    if "exec" in _CACHE:
        return _CACHE["exec"]
    from concourse import bass2jax
    nc = _build_program()

    def run(in_maps):
        return bass2jax.run_bass_via_pjrt(nc, in_maps, n_cores=N_CORES)

    _CACHE["exec"] = run
    return run


def kernel(**inputs):
    in_maps = _prep_inputs(**inputs)
    run = _get_exec()
    outs = run(in_maps)

    logits = np.empty((B * T, V), np.float32)
    sumexp = np.zeros((B * T,), np.float64)
    for c in range(N_CORES):
        lg = outs[c]["logits"]          # [B*T, VS] rows in core-major token order
        lo = c * VS
        hi = min((c + 1) * VS, V)
        if hi > lo:
            logits[:, lo:hi] = lg[:, :hi - lo]
        se = outs[c]["sumexp"]          # [128, NTG]
        sumexp += se.astype(np.float64).T.reshape(-1)

    # rows of logits are already global order: core r holds tokens r*512..
    tgt = np.asarray(inputs["targets"]).reshape(-1)
    logit_t = logits[np.arange(B * T), tgt].astype(np.float64)
    lse = np.log(sumexp)
    loss = np.float32(np.mean(lse - logit_t))
    return logits, loss


# revision 5
# speedup vs baseline: 1894.2995x; 1894.2995x over previous
"""Bass/Trainium2 kernel for nn_BigramLanguageModel: 8-core SPMD.

Sharding: tokens 8-way (core = batch_row*2 + seq_half) through embedding +
8 transformer layers with a per-pair KV AllGather per layer; vocab 8-way for
the LM head (xf AllGather over all 8 cores); per-core partial sumexp, host
combines the CE loss. Host folds LN gains into weights, casts to bf16, and
does the embedding gather.
"""
import os
import numpy as np
import ml_dtypes

import concourse.bacc as bacc
import concourse.tile as tile
from concourse import mybir
from concourse.masks import make_identity

F32 = mybir.dt.float32
BF16 = mybir.dt.bfloat16

N_CORES = 8
B, T, D, H, L_FULL, V = 4, 1024, 1024, 16, 8, 50257
DH = D // H          # 64
FF = 4 * D           # 4096
TLOC = 512           # tokens per core
NT = TLOC // 128     # 4 local token tiles
ND = D // 128        # 8
EPS = 1e-5
VS = 6656            # vocab shard per core (13*512), 8*VS = 53248 >= V
VCH = VS // 512      # 13
NTG = (B * T) // 128  # 32 global token tiles

L = int(os.environ.get("KBENCH_NLAYERS", L_FULL))

_CACHE = {}


def _build_program():
    nc = bacc.Bacc(None, target_bir_lowering=False, num_devices=N_CORES)

    x0_d = nc.dram_tensor("x0", [TLOC, D], F32, kind="ExternalInput")
    m01_d = nc.dram_tensor("m01", [NT, 128, T], BF16, kind="ExternalInput")
    vmask_d = nc.dram_tensor("vmask", [128, VCH, 512], BF16, kind="ExternalInput")
    wq_d = nc.dram_tensor("wq", [L, D, D], BF16, kind="ExternalInput")
    wk_d = nc.dram_tensor("wk", [L, D, D], BF16, kind="ExternalInput")
    wv_d = nc.dram_tensor("wv", [L, D, D], BF16, kind="ExternalInput")
    wo_d = nc.dram_tensor("wo", [L, D, D], BF16, kind="ExternalInput")
    w1_d = nc.dram_tensor("w1", [L, D, FF], BF16, kind="ExternalInput")
    w2_d = nc.dram_tensor("w2", [L, FF, D], BF16, kind="ExternalInput")
    wlm_d = nc.dram_tensor("wlm", [D, VS], BF16, kind="ExternalInput")

    logits_d = nc.dram_tensor("logits", [B * T, VS], F32, kind="ExternalOutput")
    se_d = nc.dram_tensor("sumexp", [128, NTG], F32, kind="ExternalOutput")

    with tile.TileContext(nc) as tc:
        with (
            tc.tile_pool(name="state", bufs=1) as state,
            tc.tile_pool(name="acts", bufs=1) as acts,
            tc.tile_pool(name="hbuf", bufs=2) as hbuf,
            tc.tile_pool(name="wpool", bufs=7) as wpool,
            tc.tile_pool(name="small", bufs=4) as small,
            tc.tile_pool(name="probs_p", bufs=2) as probs_p,
            tc.tile_pool(name="pt_p", bufs=2) as pt_p,
            tc.tile_pool(name="attn_p", bufs=2) as attn_p,
            tc.tile_pool(name="lgp", bufs=4) as lgp,
            tc.tile_pool(name="escp", bufs=2) as escp,
            tc.tile_pool(name="ps_g", bufs=2, space="PSUM") as ps_g,
            tc.tile_pool(name="ps_tp", bufs=2, space="PSUM") as ps_tp,
            tc.tile_pool(name="ps_at", bufs=2, space="PSUM") as ps_at,
            tc.tile_pool(name="ps_sc", bufs=2, space="PSUM") as ps_sc,
            tc.tile_pool(name="dram", bufs=2, space="DRAM") as dram,
        ):
            ident = state.tile([128, 128], BF16)
            make_identity(nc, ident)
            eps_t = state.tile([128, 1], F32)
            nc.vector.memset(eps_t, EPS)

            # resident state: x (fp32 residual), causal masks
            x = state.tile([128, NT, D], F32)
            for k in range(NT):
                nc.sync.dma_start(x[:, k], x0_d[k * 128:(k + 1) * 128, :])
            m01 = state.tile([128, NT, T], BF16)
            nc.sync.dma_start(m01, m01_d[:].rearrange("k p t -> p k t"))

            def layernorm_cast(src_f32, dst_bf16):
                """dst = (src - mean)/std per partition row, cast to bf16."""
                stats = small.tile([128, 2, 6], F32, tag="stats", name="stats")
                xg = src_f32.rearrange("p (s f) -> p s f", s=2)
                for s in range(2):
                    nc.vector.bn_stats(out=stats[:, s], in_=xg[:, s])
                mv = small.tile([128, 2], F32, tag="mv", name="mv")
                nc.vector.bn_aggr(out=mv, in_=stats)
                std = small.tile([128, 1], F32, tag="std", name="std")
                nc.scalar.activation(out=std, in_=mv[:, 1:2],
                                     func=mybir.ActivationFunctionType.Sqrt,
                                     bias=eps_t)
                rstd = small.tile([128, 1], F32, tag="rstd", name="rstd")
                nc.vector.reciprocal(out=rstd, in_=std)
                nmr = small.tile([128, 1], F32, tag="nmr", name="nmr")
                nc.vector.tensor_scalar(out=nmr, in0=mv[:, 0:1], scalar1=rstd,
                                        scalar2=-1.0,
                                        op0=mybir.AluOpType.mult,
                                        op1=mybir.AluOpType.mult)
                nc.scalar.activation(out=dst_bf16, in_=src_f32,
                                     func=mybir.ActivationFunctionType.Identity,
                                     bias=nmr, scale=rstd)

            def transpose_block(dst, src):
                """dst[128,128] (SBUF bf16) = src[128,128].T via PE+ACT."""
                pt = ps_tp.tile([128, 128], BF16, tag="tp", name="tp")
                nc.tensor.transpose(pt, src, ident)
                nc.scalar.copy(dst, pt)

            def load_w(dram_ap, tag):
                """Load [1024rows x 512cols] weight slab as [128, nsub, 512]."""
                rows = dram_ap.shape[0]
                nsub = rows // 128
                t = wpool.tile([128, nsub, 512], BF16, tag="w", name=f"w_{tag}")
                nc.sync.dma_start(t, dram_ap.rearrange("(s p) n -> p s n", p=128))
                return t

            for l in range(L):
                # ---- LN1 + h^T
                hT = acts.tile([128, ND, TLOC], BF16, tag="hT", name="hT")
                for k in range(NT):
                    h = hbuf.tile([128, D], BF16, tag="h", name="h")
                    layernorm_cast(x[:, k], h)
                    for d in range(ND):
                        transpose_block(hT[:, d, k * 128:(k + 1) * 128],
                                        h[:, d * 128:(d + 1) * 128])

                # ---- QKV projections
                wq_t = [load_w(wq_d[l, :, nh * 512:(nh + 1) * 512], "wa") for nh in range(2)]
                wk_t = [load_w(wk_d[l, :, nh * 512:(nh + 1) * 512], "wb") for nh in range(2)]
                wv_t = [load_w(wv_d[l, :, nh * 512:(nh + 1) * 512], "wc") for nh in range(2)]

                QT = acts.tile([128, ND, TLOC], BF16, tag="QT", name="QT")
                KTf = acts.tile([128, ND, T], BF16, tag="KTf", name="KTf")
                Vf = acts.tile([128, T // 128, D], BF16, tag="Vf", name="Vf")
                for (wt, dst) in ((wq_t, QT), (wk_t, None)):
                    for j in range(ND):  # output n-chunk of 128
                        ps = ps_g.tile([128, 512], F32, tag="g", name="psg")
                        for d in range(ND):
                            nc.tensor.matmul(
                                ps, wt[j // 4][:, d, (j % 4) * 128:(j % 4) * 128 + 128],
                                hT[:, d, :], start=(d == 0), stop=(d == ND - 1))
                        if dst is None:
                            nc.scalar.copy(KTf[:, j, 0:512], ps)
                        else:
                            nc.scalar.copy(dst[:, j, :], ps)
                for k in range(NT):
                    for nh in range(2):
                        ps = ps_g.tile([128, 512], F32, tag="g", name="psg")
                        for d in range(ND):
                            nc.tensor.matmul(
                                ps, hT[:, d, k * 128:(k + 1) * 128],
                                wv_t[nh][:, d, :], start=(d == 0), stop=(d == ND - 1))
                        nc.scalar.copy(Vf[:, k, nh * 512:(nh + 1) * 512], ps)

                # ---- KV AllGather within the pair
                cc_in = dram.tile([2, 128, 4096], BF16, tag="kv_in", name="kv_in")
                cc_out = dram.tile([2, 2, 128, 4096], BF16, tag="kv_out", name="kv_out")
                nc.sync.dma_start(cc_in[0].rearrange("p (s n) -> p s n", s=8),
                                  KTf[:, :, 0:512])
                nc.sync.dma_start(cc_in[1].rearrange("p (s n) -> p s n", s=4),
                                  Vf[:, 0:NT, :])
                nc.gpsimd.collective_compute(
                    "AllGather", mybir.AluOpType.bypass,
                    replica_groups=[[0, 1], [2, 3], [4, 5], [6, 7]],
                    ins=[cc_in.opt()], outs=[cc_out.opt()])
                for r in range(2):
                    nc.sync.dma_start(
                        KTf[:, :, r * 512:(r + 1) * 512],
                        cc_out[r, 0].rearrange("p (s n) -> p s n", s=8))
                    nc.sync.dma_start(
                        Vf[:, r * 4:(r + 1) * 4, :],
                        cc_out[r, 1].rearrange("p (s n) -> p s n", s=4))

                wo_t = [load_w(wo_d[l, :, nh * 512:(nh + 1) * 512], "wa") for nh in range(2)]

                # ---- attention (uniform full-span, causality via m01 data)
                attT = acts.tile([128, ND, TLOC], BF16, tag="attT", name="attT")
                for k in range(NT):
                    attn = attn_p.tile([128, H, DH], BF16, tag="attn", name="attn")
                    for hd in range(H):
                        po = (hd % 2) * 64
                        j = hd // 2
                        probs = probs_p.tile([128, T], BF16, tag="probs", name="probs")
                        for c in range(2):
                            sc = ps_sc.tile([128, 512], F32, tag="sc", name="sc")
                            nc.tensor.matmul(
                                sc, QT[po:po + 64, j, k * 128:(k + 1) * 128],
                                KTf[po:po + 64, j, c * 512:(c + 1) * 512],
                                start=True, stop=True)
                            nc.scalar.activation(
                                out=probs[:, c * 512:(c + 1) * 512], in_=sc,
                                func=mybir.ActivationFunctionType.Exp)
                        nc.vector.tensor_mul(probs, probs, m01[:, k, :])
                        se = small.tile([128, 1], F32, tag="se", name="se")
                        nc.vector.reduce_sum(se, probs, axis=mybir.AxisListType.X)
                        rcp = small.tile([128, 1], F32, tag="rcp", name="rcp")
                        nc.vector.reciprocal(out=rcp, in_=se)
                        pT = pt_p.tile([128, T // 128, 128], BF16, tag="pT", name="pT")
                        for s in range(T // 128):
                            ptp = ps_tp.tile([128, 128], BF16, tag="tp", name="tp2")
                            nc.tensor.transpose(
                                ptp, probs[:, s * 128:(s + 1) * 128], ident)
                            nc.vector.tensor_copy(pT[:, s, :], ptp)
                        pa = ps_at.tile([128, DH], F32, tag="at", name="pat")
                        for s in range(T // 128):
                            nc.tensor.matmul(
                                pa, pT[:, s, :], Vf[:, s, hd * DH:(hd + 1) * DH],
                                start=(s == 0), stop=(s == T // 128 - 1))
                        nc.vector.tensor_scalar_mul(attn[:, hd, :], in0=pa, scalar1=rcp)
                    av = attn.rearrange("p h d -> p (h d)")
                    for d in range(ND):
                        transpose_block(attT[:, d, k * 128:(k + 1) * 128],
                                        av[:, d * 128:(d + 1) * 128])

                # ---- output projection + residual
                for k in range(NT):
                    for nh in range(2):
                        ps = ps_g.tile([128, 512], F32, tag="g", name="psg")
                        for d in range(ND):
                            nc.tensor.matmul(
                                ps, attT[:, d, k * 128:(k + 1) * 128],
                                wo_t[nh][:, d, :], start=(d == 0), stop=(d == ND - 1))
                        xs = x[:, k, nh * 512:(nh + 1) * 512]
                        nc.vector.tensor_add(xs, xs, ps)

                # ---- LN2 + FFN (two 2048-halves to bound SBUF)
                h2T = acts.tile([128, ND, TLOC], BF16, tag="hT", name="h2T")
                for k in range(NT):
                    h2 = hbuf.tile([128, D], BF16, tag="h", name="h2")
                    layernorm_cast(x[:, k], h2)
                    for d in range(ND):
                        transpose_block(h2T[:, d, k * 128:(k + 1) * 128],
                                        h2[:, d * 128:(d + 1) * 128])
                for half in range(2):
                    w1_t = [load_w(w1_d[l, :, (half * 4 + q) * 512:(half * 4 + q + 1) * 512], "wa")
                            for q in range(4)]
                    uT = acts.tile([128, 16, TLOC], BF16, tag="uT", name="uT")
                    for m in range(16):  # ff chunk of 128 within this half
                        ps = ps_g.tile([128, 512], F32, tag="g", name="psg")
                        for d in range(ND):
                            nc.tensor.matmul(
                                ps, w1_t[m // 4][:, d, (m % 4) * 128:(m % 4) * 128 + 128],
                                h2T[:, d, :],
                                start=(d == 0), stop=(d == ND - 1))
                        nc.scalar.activation(out=uT[:, m, :], in_=ps,
                                             func=mybir.ActivationFunctionType.Relu)
                    w2_t = [load_w(w2_d[l, half * 2048 + g * 1024:half * 2048 + (g + 1) * 1024,
                                        nh * 512:(nh + 1) * 512], f"w2_{g}_{nh}")
                            for g in range(2) for nh in range(2)]
                    for k in range(NT):
                        for nh in range(2):
                            ps = ps_g.tile([128, 512], F32, tag="g", name="psg")
                            for s in range(16):
                                wt = w2_t[(s // 8) * 2 + nh]
                                nc.tensor.matmul(
                                    ps, uT[:, s, k * 128:(k + 1) * 128],
                                    wt[:, s % 8, :], start=(s == 0), stop=(s == 15))
                            xs = x[:, k, nh * 512:(nh + 1) * 512]
                            nc.vector.tensor_add(xs, xs, ps)

            # ---- final LN + xf AllGather
            xfT = acts.tile([128, ND, TLOC], BF16, tag="hT", name="xfT")
            for k in range(NT):
                hf = hbuf.tile([128, D], BF16, tag="h", name="hf")
                layernorm_cast(x[:, k], hf)
                for d in range(ND):
                    transpose_block(xfT[:, d, k * 128:(k + 1) * 128],
                                    hf[:, d * 128:(d + 1) * 128])
            xf_in = dram.tile([128, ND, TLOC], BF16, tag="xf_in", name="xf_in")
            xf_out = dram.tile([N_CORES, 128, ND, TLOC], BF16, tag="xf_out", name="xf_out")
            nc.sync.dma_start(xf_in, xfT)
            nc.gpsimd.collective_compute(
                "AllGather", mybir.AluOpType.bypass,
                replica_groups=[[0, 1, 2, 3, 4, 5, 6, 7]],
                ins=[xf_in.opt()], outs=[xf_out.opt()])
            # ---- LM head over the vocab shard: two vocab groups, per-core xf
            vm = acts.tile([128, VCH, 512], BF16, tag="Vf", name="vm")
            nc.sync.dma_start(vm, vmask_d[:])
            separts = state.tile([128, NTG, VCH], F32)
            for g0, g1 in ((0, 7), (7, VCH)):
                wls = [load_w(wlm_d[:, vc * 512:(vc + 1) * 512], f"wlm{vc}")
                       for vc in range(g0, g1)]
                for r in range(N_CORES):
                    xfr = acts.tile([128, ND, TLOC], BF16,
                                    tag=("QT" if r % 2 == 0 else "hT"), name="xfr")
                    nc.sync.dma_start(xfr, xf_out[r])
                    for kk in range(NT):
                        tt = r * NT + kk
                        for i, vc in enumerate(range(g0, g1)):
                            ps = ps_g.tile([128, 512], F32, tag="g", name="psg")
                            for d in range(ND):
                                nc.tensor.matmul(
                                    ps, xfr[:, d, kk * 128:(kk + 1) * 128],
                                    wls[i][:, d, :], start=(d == 0), stop=(d == ND - 1))
                            lg = lgp.tile([128, 512], F32, tag="lg", name="lg")
                            nc.scalar.copy(lg, ps)
                            esc = escp.tile([128, 512], BF16, tag="esc", name="esc")
                            nc.scalar.activation(out=esc, in_=ps,
                                                 func=mybir.ActivationFunctionType.Exp)
                            nc.vector.tensor_mul(esc, esc, vm[:, vc, :])
                            nc.vector.reduce_sum(separts[:, tt, vc:vc + 1], esc,
                                                 axis=mybir.AxisListType.X)
                            nc.sync.dma_start(
                                logits_d[tt * 128:(tt + 1) * 128,
                                         vc * 512:(vc + 1) * 512], lg)
            sesum = state.tile([128, NTG], F32)
            for tt in range(NTG):
                nc.vector.reduce_sum(sesum[:, tt:tt + 1], separts[:, tt],
                                     axis=mybir.AxisListType.X)
            nc.sync.dma_start(se_d[:], sesum)

    nc.compile()
    return nc


def _prep_inputs(idx, targets, tok_emb, pos_emb, Wq, Wk, Wv, Wo, bo,
                 ln1_g, ln1_b, ln2_g, ln2_b, W1, b1, W2, b2, lnf_g, lnf_b,
                 Wlm, blm):
    """Host-side: embedding gather, LN-gain folding, bf16 casts, masks."""
    f32 = np.float32
    bf = ml_dtypes.bfloat16
    for b_ in (bo, ln1_b, ln2_b, b1, b2, lnf_b, blm):
        assert np.max(np.abs(np.asarray(b_))) == 0.0, "nonzero bias unsupported"

    x0 = np.asarray(tok_emb, f32)[np.asarray(idx)] + np.asarray(pos_emb, f32)[None, :T]
    x0 = x0.reshape(B * T, D)

    scale = np.float32(D ** -0.5)
    g1 = np.asarray(ln1_g, f32)[:, :, None]
    g2 = np.asarray(ln2_g, f32)[:, :, None]
    wq = (np.asarray(Wq, f32) * g1 * scale).astype(bf)
    wk = (np.asarray(Wk, f32) * g1).astype(bf)
    wv = (np.asarray(Wv, f32) * g1).astype(bf)
    wo = np.asarray(Wo, f32).astype(bf)
    w1 = (np.asarray(W1, f32) * g2).astype(bf)
    w2 = np.asarray(W2, f32).astype(bf)
    wlm_full = (np.asarray(Wlm, f32) * np.asarray(lnf_g, f32)[:, None]).astype(bf)
    wlm_pad = np.zeros((D, N_CORES * VS), bf)
    wlm_pad[:, :V] = wlm_full

    in_maps = []
    for c in range(N_CORES):
        half = c % 2
        # causal 0/1 masks for this core's token tiles
        m01 = np.zeros((NT, 128, T), f32)
        for k in range(NT):
            gt = half * TLOC + k * 128 + np.arange(128)[:, None]
            m01[k] = (np.arange(T)[None, :] <= gt)
        vmask = np.zeros((VCH, 512), f32)
        base = c * VS
        for vc in range(VCH):
            cols = base + vc * 512 + np.arange(512)
            vmask[vc] = (cols < V)
        in_maps.append(dict(
            x0=np.ascontiguousarray(x0[c * TLOC:(c + 1) * TLOC]),
            m01=m01.astype(bf),
            vmask=np.broadcast_to(vmask.astype(bf), (128, VCH, 512)).copy(),
            wq=wq[:L], wk=wk[:L], wv=wv[:L], wo=wo[:L],
            w1=w1[:L], w2=w2[:L],
            wlm=np.ascontiguousarray(wlm_pad[:, c * VS:(c + 1) * VS]),
        ))
    return in_maps


class _Executor:
    """Cached jit of the SPMD program with device-resident inputs."""

    def __init__(self):
        import jax
        import numpy as _np
        from jax.experimental.shard_map import shard_map
        from jax.sharding import Mesh, PartitionSpec, NamedSharding
        from concourse import bass2jax, mybir as _mybir

        bass2jax.install_neuronx_cc_hook()
        nc = _build_program()
        self.nc = nc
        in_names, out_names, out_avals, zero_shapes = [], [], [], []
        pname = nc.partition_id_tensor.name if nc.partition_id_tensor else None
        for alloc in nc.m.functions[0].allocations:
            if not isinstance(alloc, _mybir.MemoryLocationSet):
                continue
            name = alloc.memorylocations[0].name
            if alloc.kind == "ExternalInput":
                if name != pname:
                    in_names.append(name)
            elif alloc.kind == "ExternalOutput":
                shape = tuple(alloc.tensor_shape)
                dtype = _mybir.dt.np(alloc.dtype)
                out_names.append(name)
                out_avals.append(jax.core.ShapedArray(shape, dtype))
                zero_shapes.append((shape, dtype))
        self.in_names, self.out_names = in_names, out_names
        self.zero_shapes = zero_shapes
        n_params = len(in_names)
        all_names = in_names + out_names + ([pname] if pname else [])

        def _body(*args):
            operands = list(args)
            if pname is not None:
                operands.append(bass2jax.partition_id_tensor())
            return tuple(bass2jax._bass_exec_p.bind(
                *operands,
                out_avals=tuple(out_avals),
                in_names=tuple(all_names),
                out_names=tuple(out_names),
                lowering_input_output_aliases=(),
                sim_require_finite=True,
                sim_require_nnan=True,
                nc=nc,
            ))

        devices = jax.devices()[:N_CORES]
        self.mesh = Mesh(_np.asarray(devices), ("core",))
        self.sharding = NamedSharding(self.mesh, PartitionSpec("core"))
        n_out = len(out_names)
        in_specs = (PartitionSpec("core"),) * (n_params + n_out)
        out_specs = (PartitionSpec("core"),) * n_out
        self.sharded = jax.jit(
            shard_map(_body, mesh=self.mesh, in_specs=in_specs,
                      out_specs=out_specs, check_rep=False),
            donate_argnums=tuple(range(n_params, n_params + n_out)),
            keep_unused=True,
        )
        self._jax = jax

    def upload(self, in_maps):
        import numpy as _np
        arrs = []
        for name in self.in_names:
            cat = _np.concatenate([_np.asarray(m[name]) for m in in_maps], axis=0)
            arrs.append(self._jax.device_put(cat, self.sharding))
        self._jax.block_until_ready(arrs)
        return arrs

    def make_zeros(self):
        import numpy as _np
        zs = [self._jax.device_put(
                  _np.zeros((N_CORES * s[0], *s[1:]), dt), self.sharding)
              for (s, dt) in self.zero_shapes]
        self._jax.block_until_ready(zs)
        return zs

    def execute(self, dev_inputs, zeros):
        outs = self.sharded(*dev_inputs, *zeros)
        return self._jax.block_until_ready(outs)

    def run(self, in_maps):
        import numpy as _np
        dev_inputs = self.upload(in_maps)
        outs = self.execute(dev_inputs, self.make_zeros())
        res = []
        for c in range(N_CORES):
            d = {}
            for i, name in enumerate(self.out_names):
                a = _np.asarray(outs[i])
                per = a.shape[0] // N_CORES
                d[name] = a[c * per:(c + 1) * per]
            res.append(d)
        return res


def _get_exec():
    if "exec" not in _CACHE:
        _CACHE["exec"] = _Executor()
    ex = _CACHE["exec"]
    return ex.run


def kernel(**inputs):
    in_maps = _prep_inputs(**inputs)
    run = _get_exec()
    outs = run(in_maps)

    logits = np.empty((B * T, V), np.float32)
    sumexp = np.zeros((B * T,), np.float64)
    for c in range(N_CORES):
        lg = outs[c]["logits"]          # [B*T, VS] rows in core-major token order
        lo = c * VS
        hi = min((c + 1) * VS, V)
        if hi > lo:
            logits[:, lo:hi] = lg[:, :hi - lo]
        se = outs[c]["sumexp"]          # [128, NTG]
        sumexp += se.astype(np.float64).T.reshape(-1)

    # rows of logits are already global order: core r holds tokens r*512..
    tgt = np.asarray(inputs["targets"]).reshape(-1)
    logit_t = logits[np.arange(B * T), tgt].astype(np.float64)
    lse = np.log(sumexp)
    loss = np.float32(np.mean(lse - logit_t))
    return logits, loss
